# revision 8
# baseline (speedup 1.0000x reference)
"""Trainium2 Bass kernel for 16-head causal self-attention (KaplanAttention).

Problem: x [2, 2048, 1024], torch-style weights W_q/W_k/W_v/W_o [1024, 1024].
  q/k/v = (x @ W.T) split into 16 heads of 64; causal softmax(q k^T / 8) @ v;
  concat heads; out = attn_out @ W_o.T.

Sharding (8 cores): core c handles batch b = c // 4 and head group g = c % 4
(heads 4g..4g+3). Each core computes its 4 heads' attention output and a
partial output projection against the matching 256 columns of W_o; the host
sums the 4 partials per batch (the "all-reduce" of the row-sharded W_o).

Per-core layout (everything transposed on host so the PE contraction dim is
always the partition dim; all matmul operands fp16, accumulation fp32):
  xT  [1024, 2048] = x[b].T                      e on partitions
  wqT/wkT/wvT [1024, 256] = W[256g:256g+256].T   e on partitions
  woT [256, 1024] = W_o[:, 256g:256g+256].T      d on partitions
  QT/KT [128, 2, 2048]: head pair hp, head h at partitions 64*(h%2)
  V     [128, 16, 4, 65]: j-chunk k, head h -> [V_h | ones] (ones col gives
        the softmax denominator for free from the same matmul)
  scores computed transposed: S^T[j, s]; exp on ACT (scale=1/8 fused);
  causal handled by only computing s >= 128*jt and a {0,1} upper-tri mask
  on diagonal 128x128 blocks (one DVE mul per (hp,jt), both heads at once).
  U^T is stored in four per-512-s-block tiles (block p holds rows jt<=4p+3,
  cols [max(128jt,512p), 512(p+1))) so block p can be recycled for the next
  head pair as soon as this pair's AV for t=p has consumed it.
  AV: psum [65, s] accumulates [V|1]^T @ U^T; row 64 = Z_s. Normalize: copy
  Z row to SBUF, reciprocal_approx_fast (the custom DVE op mis-addresses
  PSUM partition offsets, so it must read SBUF partition 0), gpsimd
  partition_broadcast, tensor_mul.
  Final: partial[s, m] from lhsT = normalized out^T, rhs = woT chunks;
  partials are written fp16 and summed on host in fp32.

Scheduling (v4): ACT (exp over ~4.5M score elements/core) and the PE are
both near-saturated; emission order keeps both streaming:
  - xT is DMA'd in 512-column blocks so the first Q/K tiles (and the first
    exp) unblock ~6us in, instead of after the whole 4MB load;
  - scores are emitted s-block-major, with the Q/K projection tiles for
    block p emitted just before the block's score chunks;
  - score PSUM = 2 x [128, 2, 512] (4 banks) so the next chunk's matmuls
    run under the current chunk's exp -> ACT ~100% duty;
  - one shared 4-slot 1-bank PSUM pool rotates proj/AV/final tiles;
  - V projection and hp1's Q/K tiles are emitted after hp0's scores as PE
    fillers during hp0's exp stream.
"""

import numpy as np

from concourse import bass_utils, mybir, tile
from concourse import bacc

S = 2048
D = 1024
HPC = 4        # heads per core
DK = 64
DC = HPC * DK  # 256 d-columns per core
NCORES = 8
EC = D // 128  # 8 e-chunks
NJT = S // 128  # 16 j-tiles
NST = S // 512  # 4 s 512-tiles

FP16 = mybir.dt.float16
FP32 = mybir.dt.float32

# UT block p holds, for each row-tile jt <= 4p+3, the columns
# [max(128jt, 512p), 512(p+1)).  WIDTH[p][jt] is that width and BOFF[p][jt]
# the row's offset inside the block tile.
WIDTH = [[512 * (p + 1) - max(128 * jt, 512 * p) for jt in range(4 * p + 4)]
         for p in range(NST)]
BOFF = []
for p in range(NST):
    offs, o = [], 0
    for w in WIDTH[p]:
        offs.append(o)
        o += w
    BOFF.append(offs)
WTOT = [sum(w) for w in WIDTH[p] for p in [p]] if False else [sum(ws) for ws in WIDTH]


def _build(reps=1):
    nc = bacc.Bacc("TRN2", target_bir_lowering=False, debug=False)

    xT_d = nc.dram_tensor("xT", [D, S], FP16, kind="ExternalInput")
    wq_d = nc.dram_tensor("wqT", [D, DC], FP16, kind="ExternalInput")
    wk_d = nc.dram_tensor("wkT", [D, DC], FP16, kind="ExternalInput")
    wv_d = nc.dram_tensor("wvT", [D, DC], FP16, kind="ExternalInput")
    wo_d = nc.dram_tensor("woT", [DC, D], FP16, kind="ExternalInput")
    mask_d = nc.dram_tensor("mask", [128, 2, 128], FP16, kind="ExternalInput")
    out_d = nc.dram_tensor("out", [S, D], FP16, kind="ExternalOutput")

    with tile.TileContext(nc) as tc:
        with (
            tc.tile_pool(name="const", bufs=1) as const,
            tc.tile_pool(name="work", bufs=1) as work,
            tc.tile_pool(name="ut", bufs=1) as utp,
            tc.tile_pool(name="outs", bufs=4) as outs,
            tc.tile_pool(name="norm", bufs=4) as normp,
            tc.tile_pool(name="ps1", bufs=4, space="PSUM") as ps1,
            tc.tile_pool(name="psS", bufs=2, space="PSUM") as psS,
        ):
          for _rep in range(reps):
            # ---- load inputs: Q/K weights and the mask first, then xT in
            # 512-column blocks (so block-0 Q/K tiles unblock early), then
            # the later-needed V/O weights.  DMAs are spread round-robin
            # over four engine queues — a single queue serializes them at
            # ~600ns each (~20us for the whole load). ----
            dmaq = [nc.sync, nc.scalar, nc.gpsimd]
            _qi = [0]

            def dma(out, in_):
                dmaq[_qi[0] % len(dmaq)].dma_start(out=out, in_=in_)
                _qi[0] += 1

            wq = const.tile([128, EC, DC], FP16)
            wk = const.tile([128, EC, DC], FP16)
            for w_t, w_dr in ((wq, wq_d), (wk, wk_d)):
                dma(w_t, w_dr.rearrange("(c p) d -> p c d", p=128))
            mask = const.tile([128, 2, 128], FP16)
            dma(mask, mask_d[:, :, :])
            xT = const.tile([128, EC, S], FP16)
            for st in range(NST):
                for c in range(EC):
                    dma(
                        xT[:, c, 512 * st : 512 * (st + 1)],
                        xT_d[128 * c : 128 * (c + 1), 512 * st : 512 * (st + 1)],
                    )
            wv = const.tile([128, EC, DC], FP16)
            dma(wv, wv_d.rearrange("(c p) d -> p c d", p=128))
            wo = const.tile([128, 2, D], FP16)
            dma(wo, wo_d.rearrange("(c p) d -> p c d", p=128))

            QT = work.tile([128, 2, S], FP16)
            KT = work.tile([128, 2, S], FP16)

            def qk_proj(hp, st_list):
                for st in st_list:
                    for w_t, dst in ((wq, QT), (wk, KT)):
                        ps = ps1.tile([128, 512], FP32, tag="b1", bufs=2)
                        for c in range(EC):
                            nc.tensor.matmul(
                                ps,
                                w_t[:, c, 128 * hp : 128 * (hp + 1)],
                                xT[:, c, 512 * st : 512 * (st + 1)],
                                start=(c == 0),
                                stop=(c == EC - 1),
                            )
                        nc.vector.tensor_copy(
                            out=dst[:, hp, 512 * st : 512 * (st + 1)], in_=ps
                        )

            # V tile is filled by the deferred V projection below; the ones
            # column is set once up front.
            V = work.tile([128, NJT, HPC, 65], FP16)
            nc.vector.memset(V[:, :, :, 64:65], 1.0)

            outTn = work.tile([128, 2, S], FP16)  # normalized out^T, pair-stacked

            def scores(hp, UTb, with_proj):
                for p in range(NST):
                    if with_proj:
                        qk_proj(hp, [p])
                    for jt in range(4 * p + 4):
                        s0 = 128 * jt
                        pos = max(s0, 512 * p)
                        cn = WIDTH[p][jt]
                        ps = psS.tile([128, 2, 512], FP32, tag="score")
                        for hi in range(2):
                            ho = 64 * hi
                            nc.tensor.matmul(
                                ps[:, hi, 0:cn],
                                KT[ho : ho + 64, hp, s0 : s0 + 128],
                                QT[ho : ho + 64, hp, pos : pos + cn],
                                start=True,
                                stop=True,
                            )
                        uo = BOFF[p][jt]
                        nc.scalar.activation(
                            out=UTb[p][:, :, uo : uo + cn],
                            in_=ps[:, :, 0:cn],
                            func=mybir.ActivationFunctionType.Exp,
                            scale=0.125,
                        )
                        if p == jt // 4:
                            # causal mask on the diagonal 128-block
                            nc.vector.tensor_mul(
                                UTb[p][:, :, uo : uo + 128],
                                UTb[p][:, :, uo : uo + 128],
                                mask,
                            )

            def av(hp, UTb):
                for t in range(NST):
                    for hi in range(2):
                        h = 2 * hp + hi
                        ho = 64 * hi
                        psa = ps1.tile([128, 512], FP32, tag="av", bufs=2)
                        kmax = 4 * t + 4
                        for k in range(kmax):
                            off = max(0, 128 * k - 512 * t)
                            n = 512 - off
                            uo = BOFF[t][k]
                            nc.tensor.matmul(
                                psa[0:65, off : off + n],
                                V[:, k, h, :],
                                UTb[t][:, hi, uo : uo + n],
                                start=(k == 0),
                                stop=(k == kmax - 1),
                            )
                        zc = normp.tile([1, 512], FP32, tag="zc")
                        nc.vector.tensor_copy(out=zc, in_=psa[64:65, :])
                        zr = normp.tile([1, 512], FP32, tag="zrow")
                        nc.vector.reciprocal_approx_fast(out=zr, in_=zc)
                        zb = normp.tile([64, 512], FP32, tag="zb")
                        nc.gpsimd.partition_broadcast(zb, zr)
                        nc.vector.tensor_mul(
                            outTn[ho : ho + 64, hp, 512 * t : 512 * (t + 1)],
                            psa[0:64, :],
                            zb,
                        )

            def ut_blocks():
                tiles = []
                for p in range(NST):
                    ub = utp.tile([128, 2, WTOT[p]], FP16, tag=f"ut{p}", name=f"ut{p}")
                    tiles.append(ub)
                return tiles

            # ---- hp0 pipeline ----
            UT0 = ut_blocks()
            scores(0, UT0, with_proj=True)

            # PE fillers while ACT streams hp0's exps:
            for jt in range(NJT):  # V projection
                psv = ps1.tile([128, 512], FP32, tag="b1", bufs=2)
                psd = psv[:, 0:DC]
                for c in range(EC):
                    nc.tensor.matmul(
                        psd,
                        xT[:, c, 128 * jt : 128 * (jt + 1)],
                        wv[:, c, :],
                        start=(c == 0),
                        stop=(c == EC - 1),
                    )
                nc.vector.tensor_copy(
                    out=V[:, jt, :, 0:64],
                    in_=psd.rearrange("p (h d) -> p h d", h=HPC),
                )
            av(0, UT0)
            qk_proj(1, range(NST))  # hp1's Q/K tiles

            # ---- hp1 pipeline ----
            UT1 = ut_blocks()
            scores(1, UT1, with_proj=False)
            av(1, UT1)

            # ---- final projection: partial[s, m], fp16 out ----
            for st in range(NJT):
                for mt in range(2):
                    psf = ps1.tile([128, 512], FP32, tag="b1", bufs=2)
                    for hp in range(2):
                        nc.tensor.matmul(
                            psf,
                            outTn[:, hp, 128 * st : 128 * (st + 1)],
                            wo[:, hp, 512 * mt : 512 * (mt + 1)],
                            start=(hp == 0),
                            stop=(hp == 1),
                        )
                    ob = outs.tile([128, 512], FP16, tag="ob")
                    nc.vector.tensor_copy(out=ob, in_=psf)
                    dma(
                        out_d[128 * st : 128 * (st + 1), 512 * mt : 512 * (mt + 1)],
                        ob,
                    )

    nc.compile()
    return nc


_NC = None


def _prep_in_maps(x, W_q, W_k, W_v, W_o):
    x = np.asarray(x, dtype=np.float32)
    W_q = np.asarray(W_q, dtype=np.float32)
    W_k = np.asarray(W_k, dtype=np.float32)
    W_v = np.asarray(W_v, dtype=np.float32)
    W_o = np.asarray(W_o, dtype=np.float32)
    mask01 = np.triu(np.ones((128, 128), dtype=np.float16))
    mask2 = np.ascontiguousarray(
        np.broadcast_to(mask01[:, None, :], (128, 2, 128))
    )
    in_maps = []
    for c in range(NCORES):
        b, g = divmod(c, 4)
        cols = slice(DC * g, DC * (g + 1))
        in_maps.append(
            {
                "xT": np.ascontiguousarray(x[b].T).astype(np.float16),
                "wqT": np.ascontiguousarray(W_q[cols, :].T).astype(np.float16),
                "wkT": np.ascontiguousarray(W_k[cols, :].T).astype(np.float16),
                "wvT": np.ascontiguousarray(W_v[cols, :].T).astype(np.float16),
                "woT": np.ascontiguousarray(W_o[:, cols].T).astype(np.float16),
                "mask": mask2,
            }
        )
    return in_maps


def _run(x, W_q, W_k, W_v, W_o, **spmd_kwargs):
    global _NC
    if _NC is None:
        _NC = _build()
    in_maps = _prep_in_maps(x, W_q, W_k, W_v, W_o)
    res = bass_utils.run_bass_kernel_spmd(
        _NC, in_maps, core_ids=list(range(NCORES)), **spmd_kwargs
    )
    out = np.empty((2, S, D), dtype=np.float32)
    for b in range(2):
        acc = res.results[4 * b]["out"].astype(np.float32)
        for g in range(1, 4):
            acc += res.results[4 * b + g]["out"].astype(np.float32)
        out[b] = acc
    return out, res


def kernel(x, W_q, W_k, W_v, W_o):
    out, _ = _run(x, W_q, W_k, W_v, W_o)
    return out
